# revision 2
# baseline (speedup 1.0000x reference)
"""Trainium2 Bass kernel for nn_Fast2Order_DE_Conv.

Math: out[b,o,ho,wo] = sum_{c,i,j} W[o, c*81+i*9+j] * p_i * p_j with
p_i = x[b, c, ho+di, wo+dj] (i = di*3+dj, 3x3 unfold of a 16-channel 64x64
image; output 62x62).

Algorithm: change the quadratic-feature basis from products p_i*p_j to
squares {p_i^2, (p_i+p_j)^2, i<j} (45 per channel, 720 total) and fold the
basis change into W on the host (W2 = W * M^-1).  On-chip, per spatial tile
of 512 locations:

    selection matmul (PE, f16):  s = AselT.T @ x_unfold  [768 padded rows]
    square          (ACT/DVE):   g = s^2, PSUM -> SBUF f16
    main matmul     (PE, f16):   out += W2T.T @ g, accumulated in fp32 PSUM

All matmuls use float16 (e5m10: ~f32r accuracy at half the width, 2-byte
FWL-eligible weight loads, full PE rate).  Inputs are cast to f16 on the
host so DMA loads feed the PE directly.  The 3x3 unfold itself is free: it
is expressed in the DMA access pattern (overlapping windows of the padded
l' = ho*64+wo layout).

Pipelining: tiles are software-pipelined with skew 3 (a tile's selection
matmuls + squares issue three tiles before its main matmuls) so the PE
never waits on the square engines; a burst of warmup matmuls during the
initial DMA window keeps the PE clock gate at full rate.  Per-core device
time ~55 us, ~80% of the PE streaming roofline; the residue is per-matmul
weight-load and dispatch overhead.

Sharding: data-parallel over batch, 2 batches per core on 8 cores; W-side
constants are replicated.  Output gathered by simple concatenation.
"""

import functools

import numpy as np

import concourse.bacc as bacc
import concourse.mybir as mybir
from concourse.tile import TileContext
from concourse.bass_utils import run_bass_kernel_spmd

B, C, H, WIDTH = 16, 16, 64, 64
O = 128
HO = WO = 62
N_CORES = 8
B_LOC = B // N_CORES
PAIRS = [(i, j) for i in range(9) for j in range(i, 9)]  # 45
ROW_TILES = [(0, 8), (8, 8), (16, 8), (24, 8), (32, 8), (40, 8), (48, 8), (56, 6)]
NCHUNK = 6  # g chunks of 128 rows (768 total, 48 zero-padded)
GC = 128
GH = 384  # padded g rows per c-half (360 real + 24 pad)


def _round_f32r(a: np.ndarray) -> np.ndarray:
    """Round fp32 values to the f32r grid (RNE at 12 low mantissa bits)."""
    a = np.ascontiguousarray(a, dtype=np.float32)
    bits = a.view(np.uint32).astype(np.uint64)
    half, mask = np.uint64(0x800), np.uint64(0xFFF)
    lsb = (bits >> np.uint64(12)) & np.uint64(1)
    out = ((bits + half - np.uint64(1) + lsb) & ~mask).astype(np.uint32)
    return out.view(np.float32).reshape(a.shape)


def _build_consts(Wf: np.ndarray):
    """W (128, 1296) -> (AselT [72, 360] f32, W2T [720, 128] f32, f32r grid)."""
    Wt = np.asarray(Wf, dtype=np.float64).reshape(O, C, 9, 9)
    Wsym = Wt + Wt.transpose(0, 1, 3, 2)
    W2 = np.zeros((O, 720))
    for c in range(C):
        for pi, (i, j) in enumerate(PAIRS):
            f = c * 45 + pi
            if i == j:
                W2[:, f] = Wt[:, c, i, i] - 0.5 * (
                    Wsym[:, c, i, :].sum(-1) - 2.0 * Wt[:, c, i, i]
                )
            else:
                W2[:, f] = 0.5 * Wsym[:, c, i, j]
    # x-row layout on chip: row = i*8 + c_local (i = di*3+dj kernel position)
    AselT = np.zeros((72, 384), dtype=np.float32)
    for cl in range(8):
        for pi, (i, j) in enumerate(PAIRS):
            g = cl * 45 + pi
            AselT[i * 8 + cl, g] += 1.0
            if i != j:
                AselT[j * 8 + cl, g] += 1.0
    # pad each c-half's 360 features to 384 (3 chunks of 128) so every
    # selection matmul has exactly 128 stationary columns (enables FWL)
    W2p = np.zeros((O, 768))
    W2p[:, 0:360] = W2[:, 0:360]
    W2p[:, 384:744] = W2[:, 360:720]
    W2T = np.ascontiguousarray(W2p.T).astype(np.float16)  # [768, 128]
    return AselT.astype(np.float16), W2T


def _x_window_ap(x_d, b: int, h: int, ho0: int, di: int, lt_load: int):
    """Source AP for one di of the unfold load: (dj, c, l) nesting matching
    target partitions (di*3+dj)*8 + c, free dim = padded l' = ho*64+wo."""
    ap = x_d[b, h * 8 : (h + 1) * 8, ho0 + di, 0:3].unsqueeze(-1)
    v = ap.ap
    v[0] = [1, 3]
    v[1] = [H * WIDTH, 8]
    v[2] = [1, lt_load]
    return ap


def build_nc(reps: int = 1, skew: int = 3):
    """Build the per-core program.  reps>1 wraps the body in an on-chip loop
    (used only for device-time measurement); skew is the software-pipeline
    depth between a tile's selection/squares and its main matmuls."""
    f32, f16 = mybir.dt.float32, mybir.dt.float16
    nc = bacc.Bacc("TRN2", target_bir_lowering=False)
    x_d = nc.dram_tensor("x_loc", [B_LOC, C, H, WIDTH], f16, kind="ExternalInput")
    a_d = nc.dram_tensor("aselT", [72, GH], f16, kind="ExternalInput")
    w_d = nc.dram_tensor("w2T", [2 * GH, O], f16, kind="ExternalInput")
    o_d = nc.dram_tensor("out_loc", [B_LOC, O, HO, WO], f32, kind="ExternalOutput")

    with TileContext(nc) as tc:
        with (
            tc.tile_pool(name="const", bufs=1) as cpool,
            tc.tile_pool(name="xin", bufs=2) as xpool,
            tc.tile_pool(name="gbuf", bufs=3 * (skew + 1) + 3) as gpool,
            tc.tile_pool(name="tmpbuf", bufs=4) as tmppool,
            tc.tile_pool(name="obuf", bufs=6) as opool,
            tc.tile_pool(name="ps_sel", bufs=3, space="PSUM") as pspool,
            tc.tile_pool(name="ps_out", bufs=2, space="PSUM") as popool,
        ):
            LFULL = HO * 64  # 3968 columns of the padded l' = ho*64+wo axis

            a_r = cpool.tile([72, GH], f16, tag="a_r")
            nc.sync.dma_start(a_r[:], a_d[:])

            def load_x(x_t, b, h, col0, col1, eng=None):
                """Fill x_t[:, col0:col1] of the unfold view for (b, c-half h)."""
                eng = eng or nc.sync
                for di in range(3):
                    hi = min(col1, H * WIDTH - di * 64 - 2)
                    if hi > col0:
                        ap = _x_window_ap(x_d, b, h, 0, di, hi - col0)
                        ap.offset += col0
                        eng.dma_start(x_t[di * 24 : (di + 1) * 24, col0:hi], ap)
                    if hi < col1:
                        # pad columns feed discarded outputs; fill with
                        # arbitrary valid f32r data to keep reads clean
                        eng.dma_start(
                            x_t[di * 24 : (di + 1) * 24, hi:col1],
                            _x_window_ap(x_d, b, h, 0, 0, col1 - hi),
                        )

            # all unfold loads up front; batch 0 split so tile 0 starts early
            xr_all = []
            for b in range(B_LOC):
                xr_b = []
                for h in range(2):
                    x_t = xpool.tile([72, LFULL], f16, tag=f"x{h}", name=f"x{h}_{b}")
                    xr_b.append(x_t)
                xr_all.append(xr_b)
            for h in range(2):
                load_x(xr_all[0][h], 0, h, 0, 1024)
            w_r = cpool.tile([GC, NCHUNK, O], f16, tag="w_r")
            nc.sync.dma_start(w_r[:], w_d[:].rearrange("(k p) o -> p k o", p=GC))
            for h in range(2):
                load_x(xr_all[0][h], 0, h, 1024, LFULL)
            for b in range(1, B_LOC):
                for h in range(2):
                    load_x(xr_all[b][h], b, h, 0, LFULL)

            # greedy ACT/DVE load balancing for PSUM-draining elementwise
            # ops (DVE pays double for squares: bounce + SBUF square)
            eng_busy = {"act": 0.0, "dve": 0.0}

            def square_merged(g_t, ps_s, lt):
                gv = g_t[:, :, :lt]
                pv = ps_s[:, :, :lt]
                if eng_busy["act"] + 1.0 <= eng_busy["dve"] + 2.1:
                    nc.scalar.square(gv, pv)
                    eng_busy["act"] += 1.0
                else:
                    tmp = tmppool.tile([GC, 2, 512], f32, tag="sq_tmp")
                    tv = tmp[:, :, :lt]
                    nc.vector.tensor_copy(tv, pv)
                    nc.vector.tensor_mul(gv, tv, tv)
                    eng_busy["dve"] += 2.1

            def out_copy(o_view, ps_view):
                if eng_busy["act"] + 0.9 < eng_busy["dve"] + 0.55:
                    nc.scalar.copy(o_view, ps_view)
                    eng_busy["act"] += 0.9
                else:
                    nc.vector.tensor_copy(o_view, ps_view)
                    eng_busy["dve"] += 0.55

            def do_mains(st):
                """Main matmuls + drain for a tile whose squares are issued."""
                b, ho0, nr, g_ts = st
                lt = nr * 64
                ps_o = popool.tile([O, 512], f32, tag="ps_o", name="ps_o")
                for kk in range(NCHUNK):
                    nc.tensor.matmul(
                        ps_o[:, :lt],
                        w_r[:, kk, :],
                        g_ts[kk // 2][:, kk % 2, :lt],
                        start=(kk == 0),
                        stop=(kk == NCHUNK - 1),
                    )
                # compact to [O, nr*62] so the store uses contiguous chunks
                o_t = opool.tile([O, 8 * WO], f32, tag="o", name="o_t")
                ps_view = ps_o[:, :lt].rearrange("o (r w) -> o r w", w=64)
                o_view = o_t[:, : nr * WO].rearrange("o (r w) -> o r w", w=WO)
                out_copy(o_view, ps_view[:, :, :WO])
                nc.gpsimd.dma_start(
                    o_d[b, :, ho0 : ho0 + nr, :],
                    o_t[:, : nr * WO],
                )

            # HAM warmup: keep the PE busy during the initial DMA wait so the
            # clock gate is at 8/8 when real matmuls start (dummy MMs on the
            # first tile that lands; outputs never read)
            def warmup():
                for i in range(12):
                    ps_w = popool.tile([O, 512], f32, tag="ps_o", name="warm")
                    nc.tensor.matmul(
                        ps_w[:, :360], a_r[:, :128], a_r[:, :360],
                        start=True, stop=True,
                    )

            def body(it=None, unroll=1):
                # software-pipeline skew: issue tile t's selections and
                # squares, then tile (t-skew)'s mains — squares get `skew`
                # tiles of slack before the PE needs their output
                pending = []
                for b in range(B_LOC):
                    xr = xr_all[b]
                    for ho0, nr in ROW_TILES:
                        lt = nr * 64
                        c0 = ho0 * 64
                        g_ts = []
                        for kp in range(NCHUNK // 2):
                            # two 120-row chunks share one 2-bank PSUM tile so
                            # one elementwise op drains both
                            ps_s = pspool.tile(
                                [GC, 2, 512], f32, tag="ps_s", name="ps_s"
                            )
                            for half in range(2):
                                kk = kp * 2 + half
                                h, k = divmod(kk, 3)
                                nc.tensor.matmul(
                                    ps_s[:, half, :lt],
                                    a_r[:, k * GC : (k + 1) * GC],
                                    xr[h][:, c0 : c0 + lt],
                                    start=True,
                                    stop=True,
                                )
                            g_t = gpool.tile(
                                [GC, 2, 512], f16, tag="g", name="g_t"
                            )
                            square_merged(g_t, ps_s, lt)
                            g_ts.append(g_t)
                        pending.append((b, ho0, nr, g_ts))
                        if len(pending) > skew:
                            do_mains(pending.pop(0))
                for st in pending:
                    do_mains(st)

            warmup()
            if reps == 1:
                body()
            else:
                hint = (
                    mybir.EngineType.PE,
                    mybir.EngineType.Activation,
                    mybir.EngineType.DVE,
                    mybir.EngineType.SP,
                    mybir.EngineType.Pool,
                )
                with tc.For_i(0, reps, 1, hint_engines=hint) as _it:
                    body()
    nc.compile()
    return nc


@functools.lru_cache(maxsize=1)
def _cached_nc():
    return build_nc()


def _core_inputs(x: np.ndarray, consts, k: int) -> dict:
    AselT, W2T = consts
    x_r = np.asarray(x, dtype=np.float32).astype(np.float16)
    return {
        "x_loc": np.ascontiguousarray(x_r[k * B_LOC : (k + 1) * B_LOC]),
        "aselT": AselT,
        "w2T": W2T,
    }


def kernel(x: np.ndarray, W: np.ndarray, _trace: bool = False):
    x = np.asarray(x, dtype=np.float32)
    W = np.asarray(W, dtype=np.float32)
    consts = _build_consts(W)

    nc = _cached_nc()
    in_maps = [_core_inputs(x, consts, k) for k in range(N_CORES)]
    try:
        r = run_bass_kernel_spmd(
            nc, in_maps, core_ids=list(range(N_CORES)), trace=_trace
        )
    except Exception:
        # transient NRT_EXEC_UNIT_UNRECOVERABLE has been observed once on
        # this fabric; a fresh attempt recovers
        r = run_bass_kernel_spmd(
            nc, in_maps, core_ids=list(range(N_CORES)), trace=_trace
        )
    out = np.concatenate([m["out_loc"] for m in r.results], axis=0)
    if _trace:
        kernel.last_result = r
    return out


if __name__ == "__main__":
    rng = np.random.default_rng(0)
    x = rng.standard_normal((B, C, H, WIDTH), dtype=np.float32)
    W = rng.standard_normal((O, C * 81), dtype=np.float32)
    out = kernel(x, W)
    print("out shape", out.shape, out.dtype)



# revision 15
# speedup vs baseline: 1.4572x; 1.4572x over previous
"""Trainium2 Bass kernel for nn_Fast2Order_DE_Conv.

Math: out[b,o,ho,wo] = sum_{c,i,j} W[o, c*81+i*9+j] * p_i * p_j with
p_i = x[b, c, ho+di, wo+dj] (i = di*3+dj, 3x3 unfold of a 16-channel 64x64
image; output 62x62).

Algorithm: change the quadratic-feature basis from products p_i*p_j to
squares {p_i^2, (p_i+p_j)^2, i<j} (45 per channel, 90 per channel-pair
padded to 96; 768 rows total) and fold the basis change into W on the host
(W2 = W * M^-1).  On-chip, per spatial tile of 512 padded locations:

    selection (PE, f16):   24 subtile matmul jobs [K<=18, M=32, N=512] at
                           32x32 tile_positions -- 12 concurrent sites per
                           wave, 2 waves, so the whole selection streams in
                           ~2N PE cycles instead of 6N.
    square (ACT/DVE):      g = s^2, PSUM -> SBUF f16, compacting the padded
                           l' = ho*64+wo axis to 62-wide rows.
    main matmul (PE, f16): out += W2T.T @ g over 6 x 128-row chunks,
                           N = 496 compact columns, fp32 PSUM accumulate.

The 3x3 unfold is free: it is expressed in the DMA access pattern
(overlapping windows of the padded l' layout).  x rows for channel-pair p
sit at partitions 32*(p%4) + pos*2 + cl of the wave-(p//4) x tile, so every
selection job's operands live at its tile_position's partition base.

HAM warmup: ~3.4us of sustained PE activity flips the clock gate from
1.2 GHz to 2.4 GHz, and any >3us PE-idle window flips it back.  A burst of
dummy matmuls on a memset tile (no DMA dependency) covers the ~9us DMA
start-up window; all x tiles are resident (no cross-batch buffer reuse) so
the PE never starves mid-kernel.

Sharding: data-parallel over batch, 2 batches per core on 8 cores; W-side
constants are replicated.  Output gathered by simple concatenation.
"""

import functools

import numpy as np

import concourse.bacc as bacc
import concourse.mybir as mybir
from concourse.tile import TileContext
from concourse.bass_utils import run_bass_kernel_spmd

B, C, H, WIDTH = 16, 16, 64, 64
O = 128
HO = WO = 62
N_CORES = 8
B_LOC = B // N_CORES
PAIRS = [(i, j) for i in range(9) for j in range(i, 9)]  # 45
ROW_TILES = [(0, 8), (8, 8), (16, 8), (24, 8), (32, 8), (40, 8), (48, 8), (56, 6)]
NCHUNK = 6  # main-matmul contraction chunks of 128 rows (768 = 8 pairs x 96)
GC = 128
LFULL = HO * 64  # 3968 padded columns of l' = ho*64+wo


def _build_consts(Wf: np.ndarray):
    """W (128, 1296) -> (A [128, 192] f16 selection blocks, W2T [768, 128] f16).

    Feature row F = 96*pair + 45*cl + pf for channel c = 2*pair + cl and
    squares-basis feature pf (45 per channel; rows 90..96 of each pair are
    zero padding).  A is stored wave-major in columns: wave w block at
    cols [96w, 96w+96), rows 32*(pair%4) + di*6 + cl*3 + dj (one contiguous
    6-partition run per (li, di) for the unfold DMA).
    """
    Wt = np.asarray(Wf, dtype=np.float64).reshape(O, C, 9, 9)
    Wsym = Wt + Wt.transpose(0, 1, 3, 2)
    W2T = np.zeros((2 * GC * 3, O))
    A = np.zeros((GC, 2 * 96), dtype=np.float32)
    for pair in range(8):
        w, li = divmod(pair, 4)
        for cl in range(2):
            c = 2 * pair + cl
            for pf, (i, j) in enumerate(PAIRS):
                F = 96 * pair + 45 * cl + pf
                if i == j:
                    W2T[F] = Wt[:, c, i, i] - 0.5 * (
                        Wsym[:, c, i, :].sum(-1) - 2.0 * Wt[:, c, i, i]
                    )
                else:
                    W2T[F] = 0.5 * Wsym[:, c, i, j]
                for pos in {i, j}:
                    di, dj = divmod(pos, 3)
                    A[32 * li + di * 6 + cl * 3 + dj, 96 * w + 45 * cl + pf] += 1.0
    return A.astype(np.float16), np.ascontiguousarray(W2T).astype(np.float16)


def _x_pair_ap(x_d, b: int, w: int, li: int, di: int, lt_load: int):
    """Source AP for the (li, di) unfold load of wave w: dims (cl, dj, l)
    matching target partitions 32*li + di*6 + cl*3 + dj (a contiguous
    6-partition run), free dim = padded l' = ho*64+wo."""
    c0 = 8 * w + 2 * li
    ap = x_d[b, c0 : c0 + 2, di, 0:3].unsqueeze(-1)
    v = ap.ap
    v[0] = [H * WIDTH, 2]  # cl
    v[1] = [1, 3]  # dj
    v[2] = [1, lt_load]  # l'
    return ap


def build_nc(reps: int = 1, skew: int = 2, warmups: int = 24):
    """Build the per-core program.  reps>1 wraps the compute body in an
    on-chip loop (loads stay outside; used for loop timing only)."""
    f32, f16 = mybir.dt.float32, mybir.dt.float16
    nc = bacc.Bacc("TRN2", target_bir_lowering=False)
    x_d = nc.dram_tensor("x_loc", [B_LOC, C, H, WIDTH], f16, kind="ExternalInput")
    a_d = nc.dram_tensor("aselT", [GC, 2 * 96], f16, kind="ExternalInput")
    w_d = nc.dram_tensor("w2T", [NCHUNK * GC, O], f16, kind="ExternalInput")
    o_d = nc.dram_tensor("out_loc", [B_LOC, O, HO, WO], f32, kind="ExternalOutput")

    with TileContext(nc) as tc:
        with (
            tc.tile_pool(name="const", bufs=1) as cpool,
            tc.tile_pool(name="xin", bufs=2 * B_LOC) as xpool,
            tc.tile_pool(name="gbuf", bufs=3 * (skew + 1) + 3) as gpool,
            tc.tile_pool(name="tmpbuf", bufs=4) as tmppool,
            tc.tile_pool(name="obuf", bufs=6) as opool,
            tc.tile_pool(name="ps_sel", bufs=3, space="PSUM") as pspool,
            tc.tile_pool(name="ps_out", bufs=2, space="PSUM") as popool,
        ):
            # warmup tile first: memset has no DMA dependency, so dummy
            # matmuls can start immediately and hold the HAM clock gate
            # open through the DMA-engine start-up window
            wt = cpool.tile([GC, 512], f16, tag="warm")
            nc.vector.memset(wt[:], 0.0)

            a_r = cpool.tile([GC, 2 * 96], f16, tag="a_r")
            nc.sync.dma_start(a_r[:], a_d[:])

            def load_x(x_t, b, w, eng):
                """Fill the unfold view for (b, wave w): one DMA per (li, di)
                targeting a contiguous 6-partition run.  Columns beyond the
                valid source window (last 2 of di=2) stay unwritten -- they
                map to wo>=62 pad outputs that the compact squares never
                read."""
                for li in range(4):
                    for di in range(3):
                        hi = min(LFULL, H * WIDTH - di * 64 - 2)
                        p0 = 32 * li + di * 6
                        eng.dma_start(
                            x_t[p0 : p0 + 6, 0:hi],
                            _x_pair_ap(x_d, b, w, li, di, hi),
                        )

            xr_all = []
            for b in range(B_LOC):
                xr_b = []
                for w in range(2):
                    x_t = xpool.tile([GC, LFULL], f16, tag=f"x{w}", name=f"x{w}_{b}")
                    xr_b.append(x_t)
                xr_all.append(xr_b)
            # batch-0 waves first on separate DMA queues (sync + scalar so
            # scalar's issue burst finishes before its first square), then
            # weights, then batch 1 (sync + gpsimd)
            w_r = cpool.tile([GC, NCHUNK, O], f16, tag="w_r")
            load_x(xr_all[0][0], 0, 0, nc.sync)
            load_x(xr_all[0][1], 0, 1, nc.scalar)
            nc.sync.dma_start(w_r[:], w_d[:].rearrange("(k p) o -> p k o", p=GC))
            for b in range(1, B_LOC):
                load_x(xr_all[b][0], b, 0, nc.sync)
                load_x(xr_all[b][1], b, 1, nc.gpsimd)

            # greedy ACT/DVE load balancing for PSUM-draining elementwise
            # ops (weights ~ measured us per op: ACT square 1.0 / copy 0.62,
            # DVE square (f16 bounce + mul) 1.75 / copy 0.7)
            eng_busy = {"act": 0.0, "dve": 0.0}

            def square_merged(g_t, ps_s, nr):
                lt, ltc = nr * 64, nr * WO
                pv = ps_s[:, :, :lt].rearrange("p h (r w) -> p h r w", w=64)
                pv = pv[:, :, :, :WO]
                gv = g_t[:, :, :ltc].rearrange("p h (r w) -> p h r w", w=WO)
                if eng_busy["act"] + 1.0 <= eng_busy["dve"] + 1.75:
                    nc.scalar.square(gv, pv)
                    eng_busy["act"] += 1.0
                else:
                    tmp = tmppool.tile([GC, 2, 496], f16, tag="sq_tmp")
                    tv = tmp[:, :, :ltc].rearrange("p h (r w) -> p h r w", w=WO)
                    nc.vector.tensor_copy(tv, pv)
                    nc.vector.tensor_mul(
                        g_t[:, :, :ltc], tmp[:, :, :ltc], tmp[:, :, :ltc]
                    )
                    eng_busy["dve"] += 1.75

            def out_copy(o_view, ps_view):
                if eng_busy["act"] + 0.62 < eng_busy["dve"] + 0.7:
                    nc.scalar.copy(o_view, ps_view)
                    eng_busy["act"] += 0.62
                else:
                    nc.vector.tensor_copy(o_view, ps_view)
                    eng_busy["dve"] += 0.7

            def do_mains(st):
                """Main matmuls + drain for a tile whose squares are issued."""
                b, ho0, nr, g_ts = st
                ltc = nr * WO
                ps_o = popool.tile([O, 496], f32, tag="ps_o", name="ps_o")
                for kk in range(NCHUNK):
                    nc.tensor.matmul(
                        ps_o[:, :ltc],
                        w_r[:, kk, :],
                        g_ts[kk // 2][:, kk % 2, :ltc],
                        start=(kk == 0),
                        stop=(kk == NCHUNK - 1),
                    )
                o_t = opool.tile([O, 8 * WO], f32, tag="o", name="o_t")
                out_copy(o_t[:, :ltc], ps_o[:, :ltc])
                nc.gpsimd.dma_start(
                    o_d[b, :, ho0 : ho0 + nr, :],
                    o_t[:, :ltc],
                )

            def warmup():
                for _ in range(warmups):
                    ps_w = popool.tile([O, 496], f32, tag="ps_o", name="warm")
                    nc.tensor.matmul(
                        ps_w[:], wt[:, :GC], wt[:, :496], start=True, stop=True
                    )

            def sel_wave(xr, w, ps_ts, c0, lt):
                """Issue wave w's 12 selection subtile jobs for one column
                tile: channel-pairs 4w..4w+4, sites (32*li, F0%128)."""
                for li in range(4):
                    pair = 4 * w + li
                    rhs = xr[w][32 * li : 32 * li + 18, c0 : c0 + lt]
                    for s in range(3):
                        F0 = 96 * pair + 32 * s
                        ck, po = divmod(F0, GC)
                        nc.tensor.matmul(
                            ps_ts[ck // 2][po : po + 32, ck % 2, :lt],
                            a_r[32 * li : 32 * li + 18, 96 * w + 32 * s : 96 * w + 32 * s + 32],
                            rhs,
                            start=True,
                            stop=True,
                            tile_position=(32 * li, po),
                        )

            def body(it=None, unroll=1):
                pending = []
                for b in range(B_LOC):
                    xr = xr_all[b]
                    for ho0, nr in ROW_TILES:
                        lt = nr * 64
                        c0 = ho0 * 64
                        ps_ts = [
                            pspool.tile([GC, 2, 512], f32, tag="ps_s", name="ps_s")
                            for _ in range(3)
                        ]
                        g_ts = [
                            gpool.tile([GC, 2, 496], f16, tag="g", name="g_t")
                            for _ in range(3)
                        ]
                        sel_wave(xr, 0, ps_ts, c0, lt)
                        # psum tile 0 (chunks 0,1) is complete after wave 0:
                        # drain it while wave 1 streams
                        square_merged(g_ts[0], ps_ts[0], nr)
                        sel_wave(xr, 1, ps_ts, c0, lt)
                        square_merged(g_ts[1], ps_ts[1], nr)
                        square_merged(g_ts[2], ps_ts[2], nr)
                        pending.append((b, ho0, nr, g_ts))
                        if len(pending) > skew:
                            do_mains(pending.pop(0))
                for st in pending:
                    do_mains(st)

            warmup()
            if reps == 1:
                body()
            else:
                hint = (
                    mybir.EngineType.PE,
                    mybir.EngineType.Activation,
                    mybir.EngineType.DVE,
                    mybir.EngineType.SP,
                    mybir.EngineType.Pool,
                )
                with tc.For_i(0, reps, 1, hint_engines=hint) as _it:
                    body()
    nc.compile()
    return nc


@functools.lru_cache(maxsize=1)
def _cached_nc():
    return build_nc()


def _core_inputs(x: np.ndarray, consts, k: int) -> dict:
    A, W2T = consts
    x_r = np.asarray(x, dtype=np.float32).astype(np.float16)
    return {
        "x_loc": np.ascontiguousarray(x_r[k * B_LOC : (k + 1) * B_LOC]),
        "aselT": A,
        "w2T": W2T,
    }


def kernel(x: np.ndarray, W: np.ndarray, _trace: bool = False):
    x = np.asarray(x, dtype=np.float32)
    W = np.asarray(W, dtype=np.float32)
    consts = _build_consts(W)

    nc = _cached_nc()
    in_maps = [_core_inputs(x, consts, k) for k in range(N_CORES)]
    try:
        r = run_bass_kernel_spmd(
            nc, in_maps, core_ids=list(range(N_CORES)), trace=_trace
        )
    except Exception:
        # transient NRT_EXEC_UNIT_UNRECOVERABLE has been observed once on
        # this fabric; a fresh attempt recovers
        r = run_bass_kernel_spmd(
            nc, in_maps, core_ids=list(range(N_CORES)), trace=_trace
        )
    out = np.concatenate([m["out_loc"] for m in r.results], axis=0)
    if _trace:
        kernel.last_result = r
    return out


if __name__ == "__main__":
    rng = np.random.default_rng(0)
    x = rng.standard_normal((B, C, H, WIDTH), dtype=np.float32)
    W = rng.standard_normal((O, C * 81), dtype=np.float32)
    out = kernel(x, W)
    print("out shape", out.shape, out.dtype)


# revision 22
# speedup vs baseline: 1.7878x; 1.2268x over previous
"""Trainium2 Bass kernel for nn_Fast2Order_DE_Conv.

Math: out[b,o,ho,wo] = sum_{c,i,j} W[o, c*81+i*9+j] * p_i * p_j with
p_i = x[b, c, ho+di, wo+dj] (i = di*3+dj, 3x3 unfold of a 16-channel 64x64
image; output 62x62).

Algorithm: change the quadratic-feature basis from products p_i*p_j to
squares {p_i^2, (p_i+p_j)^2, i<j} (45 per channel, 720 total) and fold the
basis change into W on the host (W2 = W * M^-1).  On-chip, per spatial tile
of 512 locations:

    selection matmul (PE, f16):  s = AselT.T @ x_unfold  [768 padded rows]
    square          (ACT/DVE):   g = s^2, PSUM -> SBUF f16
    main matmul     (PE, f16):   out += W2T.T @ g, accumulated in fp32 PSUM

All matmuls use float16 (e5m10: ~f32r accuracy at half the width, 2-byte
FWL-eligible weight loads, full PE rate).  Inputs are cast to f16 on the
host so DMA loads feed the PE directly.  The 3x3 unfold itself is free: it
is expressed in the DMA access pattern (overlapping windows of the padded
l' = ho*64+wo layout).

Pipelining: tiles are software-pipelined with skew 3 (a tile's selection
matmuls + squares issue three tiles before its main matmuls) so the PE
never waits on the square engines; a burst of warmup matmuls during the
initial DMA window keeps the PE clock gate at full rate.  Per-core device
time ~55 us, ~80% of the PE streaming roofline; the residue is per-matmul
weight-load and dispatch overhead.

Sharding: data-parallel over batch, 2 batches per core on 8 cores; W-side
constants are replicated.  Output gathered by simple concatenation.
"""

import functools

import numpy as np

import concourse.bacc as bacc
import concourse.mybir as mybir
from concourse.tile import TileContext
from concourse.bass_utils import run_bass_kernel_spmd

B, C, H, WIDTH = 16, 16, 64, 64
O = 128
HO = WO = 62
N_CORES = 8
B_LOC = B // N_CORES
PAIRS = [(i, j) for i in range(9) for j in range(i, 9)]  # 45
ROW_TILES = [(0, 8), (8, 8), (16, 8), (24, 8), (32, 8), (40, 8), (48, 8), (56, 6)]
NCHUNK = 6  # g chunks of 128 rows (768 total, 48 zero-padded)
GC = 128
GH = 384  # padded g rows per c-half (360 real + 24 pad)


def _round_f32r(a: np.ndarray) -> np.ndarray:
    """Round fp32 values to the f32r grid (RNE at 12 low mantissa bits)."""
    a = np.ascontiguousarray(a, dtype=np.float32)
    bits = a.view(np.uint32).astype(np.uint64)
    half, mask = np.uint64(0x800), np.uint64(0xFFF)
    lsb = (bits >> np.uint64(12)) & np.uint64(1)
    out = ((bits + half - np.uint64(1) + lsb) & ~mask).astype(np.uint32)
    return out.view(np.float32).reshape(a.shape)


def _build_consts(Wf: np.ndarray):
    """W (128, 1296) -> (AselT [72, 360] f32, W2T [720, 128] f32, f32r grid)."""
    Wt = np.asarray(Wf, dtype=np.float64).reshape(O, C, 9, 9)
    Wsym = Wt + Wt.transpose(0, 1, 3, 2)
    W2 = np.zeros((O, 720))
    for c in range(C):
        for pi, (i, j) in enumerate(PAIRS):
            f = c * 45 + pi
            if i == j:
                W2[:, f] = Wt[:, c, i, i] - 0.5 * (
                    Wsym[:, c, i, :].sum(-1) - 2.0 * Wt[:, c, i, i]
                )
            else:
                W2[:, f] = 0.5 * Wsym[:, c, i, j]
    # x-row layout on chip: row = i*8 + c_local (i = di*3+dj kernel position)
    AselT = np.zeros((72, 384), dtype=np.float32)
    for cl in range(8):
        for pi, (i, j) in enumerate(PAIRS):
            g = cl * 45 + pi
            AselT[i * 8 + cl, g] += 1.0
            if i != j:
                AselT[j * 8 + cl, g] += 1.0
    # pad each c-half's 360 features to 384 (3 chunks of 128) so every
    # selection matmul has exactly 128 stationary columns (enables FWL)
    W2p = np.zeros((O, 768))
    W2p[:, 0:360] = W2[:, 0:360]
    W2p[:, 384:744] = W2[:, 360:720]
    W2T = np.ascontiguousarray(W2p.T).astype(np.float16)  # [768, 128]
    return AselT.astype(np.float16), W2T


def _x_window_ap(x_d, b: int, h: int, ho0: int, di: int, lt_load: int):
    """Source AP for one di of the unfold load: (dj, c, l) nesting matching
    target partitions (di*3+dj)*8 + c, free dim = padded l' = ho*64+wo."""
    ap = x_d[b, h * 8 : (h + 1) * 8, ho0 + di, 0:3].unsqueeze(-1)
    v = ap.ap
    v[0] = [1, 3]
    v[1] = [H * WIDTH, 8]
    v[2] = [1, lt_load]
    return ap


def build_nc(reps: int = 1, skew: int = 2, warmups: int = 36):
    """Build the per-core program.  reps>1 wraps the body in an on-chip loop
    (used only for device-time measurement); skew is the software-pipeline
    depth between a tile's selection/squares and its main matmuls."""
    f32, f16 = mybir.dt.float32, mybir.dt.float16
    nc = bacc.Bacc("TRN2", target_bir_lowering=False)
    x_d = nc.dram_tensor("x_loc", [B_LOC, C, H, WIDTH], f16, kind="ExternalInput")
    a_d = nc.dram_tensor("aselT", [72, GH], f16, kind="ExternalInput")
    w_d = nc.dram_tensor("w2T", [2 * GH, O], f16, kind="ExternalInput")
    o_d = nc.dram_tensor("out_loc", [B_LOC, O, HO, WO], f32, kind="ExternalOutput")

    with TileContext(nc) as tc:
        with (
            tc.tile_pool(name="const", bufs=1) as cpool,
            tc.tile_pool(name="xin", bufs=2 * B_LOC) as xpool,
            tc.tile_pool(name="gbuf", bufs=3 * (skew + 1) + 3) as gpool,
            tc.tile_pool(name="tmpbuf", bufs=4) as tmppool,
            tc.tile_pool(name="obuf", bufs=6) as opool,
            tc.tile_pool(name="ps_sel", bufs=3, space="PSUM") as pspool,
            tc.tile_pool(name="ps_out", bufs=2, space="PSUM") as popool,
        ):
            LFULL = HO * 64  # 3968 columns of the padded l' = ho*64+wo axis

            # warmup tile first: memset has no DMA dependency, so dummy
            # matmuls can start immediately and hold the HAM clock gate
            # open (2.4 GHz) through the ~9us DMA-engine start-up window
            wt = cpool.tile([GC, 512], f16, tag="warm")
            nc.vector.memset(wt[:], 0.0)

            a_r = cpool.tile([72, GH], f16, tag="a_r")
            nc.sync.dma_start(a_r[:], a_d[:])

            def load_x(x_t, b, h, col0, col1, eng=None):
                """Fill x_t[:, col0:col1] of the unfold view for (b, c-half h)."""
                eng = eng or nc.sync
                for di in range(3):
                    hi = min(col1, H * WIDTH - di * 64 - 2)
                    if hi > col0:
                        ap = _x_window_ap(x_d, b, h, 0, di, hi - col0)
                        ap.offset += col0
                        eng.dma_start(x_t[di * 24 : (di + 1) * 24, col0:hi], ap)
                    if hi < col1:
                        # pad columns feed discarded outputs; fill with
                        # arbitrary valid f32r data to keep reads clean
                        eng.dma_start(
                            x_t[di * 24 : (di + 1) * 24, hi:col1],
                            _x_window_ap(x_d, b, h, 0, 0, col1 - hi),
                        )

            # all unfold loads up front; batch 0 split so tile 0 starts
            # early; the two halves go to different DMA queues (sync /
            # scalar) and batch 1's second half to gpsimd so transfers
            # overlap across engine sets.  All four x tiles are resident
            # (bufs=4), so batch 1 never waits on batch 0's buffers.
            xr_all = []
            for b in range(B_LOC):
                xr_b = []
                for h in range(2):
                    x_t = xpool.tile([72, LFULL], f16, tag=f"x{h}", name=f"x{h}_{b}")
                    xr_b.append(x_t)
                xr_all.append(xr_b)
            load_x(xr_all[0][0], 0, 0, 0, 1024, nc.sync)
            load_x(xr_all[0][1], 0, 1, 0, 1024, nc.scalar)
            w_r = cpool.tile([GC, NCHUNK, O], f16, tag="w_r")
            nc.sync.dma_start(w_r[:], w_d[:].rearrange("(k p) o -> p k o", p=GC))
            load_x(xr_all[0][0], 0, 0, 1024, LFULL, nc.sync)
            load_x(xr_all[0][1], 0, 1, 1024, LFULL, nc.scalar)
            for b in range(1, B_LOC):
                load_x(xr_all[b][0], b, 0, 0, LFULL, nc.sync)
                load_x(xr_all[b][1], b, 1, 0, LFULL, nc.gpsimd)

            # greedy ACT/DVE load balancing for PSUM-draining elementwise
            # ops (DVE pays double for squares: bounce + SBUF square)
            eng_busy = {"act": 0.0, "dve": 0.0}

            def square_merged(g_t, ps_s, lt):
                gv = g_t[:, :, :lt]
                pv = ps_s[:, :, :lt]
                if eng_busy["act"] + 1.0 <= eng_busy["dve"] + 1.75:
                    nc.scalar.square(gv, pv)
                    eng_busy["act"] += 1.0
                else:
                    # f16 bounce: PSUM f32 read is 1x either way, but the
                    # f16 x f16 multiply runs at DVE 2x rate
                    tmp = tmppool.tile([GC, 2, 512], f16, tag="sq_tmp")
                    tv = tmp[:, :, :lt]
                    nc.vector.tensor_copy(tv, pv)
                    nc.vector.tensor_mul(gv, tv, tv)
                    eng_busy["dve"] += 1.75

            def out_copy(o_view, ps_view):
                if eng_busy["act"] + 0.9 < eng_busy["dve"] + 0.55:
                    nc.scalar.copy(o_view, ps_view)
                    eng_busy["act"] += 0.9
                else:
                    nc.vector.tensor_copy(o_view, ps_view)
                    eng_busy["dve"] += 0.55

            def do_mains(st):
                """Main matmuls + drain for a tile whose squares are issued."""
                b, ho0, nr, g_ts = st
                lt = nr * 64
                ps_o = popool.tile([O, 512], f32, tag="ps_o", name="ps_o")
                for kk in range(NCHUNK):
                    nc.tensor.matmul(
                        ps_o[:, :lt],
                        w_r[:, kk, :],
                        g_ts[kk // 2][:, kk % 2, :lt],
                        start=(kk == 0),
                        stop=(kk == NCHUNK - 1),
                    )
                # compact to [O, nr*62] so the store uses contiguous chunks
                o_t = opool.tile([O, 8 * WO], f32, tag="o", name="o_t")
                ps_view = ps_o[:, :lt].rearrange("o (r w) -> o r w", w=64)
                o_view = o_t[:, : nr * WO].rearrange("o (r w) -> o r w", w=WO)
                out_copy(o_view, ps_view[:, :, :WO])
                nc.gpsimd.dma_start(
                    o_d[b, :, ho0 : ho0 + nr, :],
                    o_t[:, : nr * WO],
                )

            # HAM warmup: keep the PE busy from t=0 through the DMA start-up
            # window so the clock gate is at 8/8 when real matmuls start
            # (dummy MMs on a memset tile; outputs never read)
            def warmup():
                for i in range(warmups):
                    ps_w = popool.tile([O, 512], f32, tag="ps_o", name="warm")
                    nc.tensor.matmul(
                        ps_w[:], wt[:, :GC], wt[:], start=True, stop=True
                    )

            def body(it=None, unroll=1):
                # software-pipeline skew: issue tile t's selections and
                # squares, then tile (t-skew)'s mains — squares get `skew`
                # tiles of slack before the PE needs their output
                pending = []
                for b in range(B_LOC):
                    xr = xr_all[b]
                    for ho0, nr in ROW_TILES:
                        lt = nr * 64
                        c0 = ho0 * 64
                        g_ts = []
                        for kp in range(NCHUNK // 2):
                            # two 120-row chunks share one 2-bank PSUM tile so
                            # one elementwise op drains both
                            ps_s = pspool.tile(
                                [GC, 2, 512], f32, tag="ps_s", name="ps_s"
                            )
                            for half in range(2):
                                kk = kp * 2 + half
                                h, k = divmod(kk, 3)
                                nc.tensor.matmul(
                                    ps_s[:, half, :lt],
                                    a_r[:, k * GC : (k + 1) * GC],
                                    xr[h][:, c0 : c0 + lt],
                                    start=True,
                                    stop=True,
                                )
                            g_t = gpool.tile(
                                [GC, 2, 512], f16, tag="g", name="g_t"
                            )
                            square_merged(g_t, ps_s, lt)
                            g_ts.append(g_t)
                        pending.append((b, ho0, nr, g_ts))
                        if len(pending) > skew:
                            do_mains(pending.pop(0))
                for st in pending:
                    do_mains(st)

            warmup()
            if reps == 1:
                body()
            else:
                hint = (
                    mybir.EngineType.PE,
                    mybir.EngineType.Activation,
                    mybir.EngineType.DVE,
                    mybir.EngineType.SP,
                    mybir.EngineType.Pool,
                )
                with tc.For_i(0, reps, 1, hint_engines=hint) as _it:
                    body()
    nc.compile()
    return nc


@functools.lru_cache(maxsize=1)
def _cached_nc():
    return build_nc()


def _core_inputs(x: np.ndarray, consts, k: int) -> dict:
    AselT, W2T = consts
    x_r = np.asarray(x, dtype=np.float32).astype(np.float16)
    return {
        "x_loc": np.ascontiguousarray(x_r[k * B_LOC : (k + 1) * B_LOC]),
        "aselT": AselT,
        "w2T": W2T,
    }


def kernel(x: np.ndarray, W: np.ndarray, _trace: bool = False):
    x = np.asarray(x, dtype=np.float32)
    W = np.asarray(W, dtype=np.float32)
    consts = _build_consts(W)

    nc = _cached_nc()
    in_maps = [_core_inputs(x, consts, k) for k in range(N_CORES)]
    try:
        r = run_bass_kernel_spmd(
            nc, in_maps, core_ids=list(range(N_CORES)), trace=_trace
        )
    except Exception:
        # transient NRT_EXEC_UNIT_UNRECOVERABLE has been observed once on
        # this fabric; a fresh attempt recovers
        r = run_bass_kernel_spmd(
            nc, in_maps, core_ids=list(range(N_CORES)), trace=_trace
        )
    out = np.concatenate([m["out_loc"] for m in r.results], axis=0)
    if _trace:
        kernel.last_result = r
    return out


if __name__ == "__main__":
    rng = np.random.default_rng(0)
    x = rng.standard_normal((B, C, H, WIDTH), dtype=np.float32)
    W = rng.standard_normal((O, C * 81), dtype=np.float32)
    out = kernel(x, W)
    print("out shape", out.shape, out.dtype)



# revision 24
# speedup vs baseline: 1.8025x; 1.0083x over previous
"""Trainium2 Bass kernel for nn_Fast2Order_DE_Conv.

Math: out[b,o,ho,wo] = sum_{c,i,j} W[o, c*81+i*9+j] * p_i * p_j with
p_i = x[b, c, ho+di, wo+dj] (i = di*3+dj, 3x3 unfold of a 16-channel 64x64
image; output 62x62).

Algorithm: change the quadratic-feature basis from products p_i*p_j to
squares {p_i^2, (p_i+p_j)^2, i<j} (45 per channel, 720 total) and fold the
basis change into W on the host (W2 = W * M^-1).  On-chip, per spatial tile
of 512 locations:

    selection matmul (PE, f16):  s = AselT.T @ x_unfold  [768 padded rows]
    square          (ACT/DVE):   g = s^2, PSUM -> SBUF f16
    main matmul     (PE, f16):   out += W2T.T @ g, accumulated in fp32 PSUM

All matmuls use float16 (e5m10: ~f32r accuracy at half the width, 2-byte
FWL-eligible weight loads, full PE rate).  Inputs are cast to f16 on the
host so DMA loads feed the PE directly.  The 3x3 unfold itself is free: it
is expressed in the DMA access pattern (overlapping windows of the padded
l' = ho*64+wo layout).

Pipelining: tiles are software-pipelined with skew 3 (a tile's selection
matmuls + squares issue three tiles before its main matmuls) so the PE
never waits on the square engines; a burst of warmup matmuls during the
initial DMA window keeps the PE clock gate at full rate.  Per-core device
time ~55 us, ~80% of the PE streaming roofline; the residue is per-matmul
weight-load and dispatch overhead.

Sharding: data-parallel over batch, 2 batches per core on 8 cores; W-side
constants are replicated.  Output gathered by simple concatenation.
"""

import functools

import numpy as np

import concourse.bacc as bacc
import concourse.mybir as mybir
from concourse.tile import TileContext
from concourse.bass_utils import run_bass_kernel_spmd

B, C, H, WIDTH = 16, 16, 64, 64
O = 128
HO = WO = 62
N_CORES = 8
B_LOC = B // N_CORES
PAIRS = [(i, j) for i in range(9) for j in range(i, 9)]  # 45
ROW_TILES = [(0, 8), (8, 8), (16, 8), (24, 8), (32, 8), (40, 8), (48, 8), (56, 6)]
NCHUNK = 6  # g chunks of 128 rows (768 total, 48 zero-padded)
GC = 128
GH = 384  # padded g rows per c-half (360 real + 24 pad)


def _round_f32r(a: np.ndarray) -> np.ndarray:
    """Round fp32 values to the f32r grid (RNE at 12 low mantissa bits)."""
    a = np.ascontiguousarray(a, dtype=np.float32)
    bits = a.view(np.uint32).astype(np.uint64)
    half, mask = np.uint64(0x800), np.uint64(0xFFF)
    lsb = (bits >> np.uint64(12)) & np.uint64(1)
    out = ((bits + half - np.uint64(1) + lsb) & ~mask).astype(np.uint32)
    return out.view(np.float32).reshape(a.shape)


def _build_consts(Wf: np.ndarray):
    """W (128, 1296) -> (AselT [72, 360] f32, W2T [720, 128] f32, f32r grid)."""
    Wt = np.asarray(Wf, dtype=np.float64).reshape(O, C, 9, 9)
    Wsym = Wt + Wt.transpose(0, 1, 3, 2)
    W2 = np.zeros((O, 720))
    for c in range(C):
        for pi, (i, j) in enumerate(PAIRS):
            f = c * 45 + pi
            if i == j:
                W2[:, f] = Wt[:, c, i, i] - 0.5 * (
                    Wsym[:, c, i, :].sum(-1) - 2.0 * Wt[:, c, i, i]
                )
            else:
                W2[:, f] = 0.5 * Wsym[:, c, i, j]
    # x-row layout on chip: row = i*8 + c_local (i = di*3+dj kernel position)
    AselT = np.zeros((72, 384), dtype=np.float32)
    for cl in range(8):
        for pi, (i, j) in enumerate(PAIRS):
            g = cl * 45 + pi
            AselT[i * 8 + cl, g] += 1.0
            if i != j:
                AselT[j * 8 + cl, g] += 1.0
    # pad each c-half's 360 features to 384 (3 chunks of 128) so every
    # selection matmul has exactly 128 stationary columns (enables FWL)
    W2p = np.zeros((O, 768))
    W2p[:, 0:360] = W2[:, 0:360]
    W2p[:, 384:744] = W2[:, 360:720]
    W2T = np.ascontiguousarray(W2p.T).astype(np.float16)  # [768, 128]
    return AselT.astype(np.float16), W2T


def _x_window_ap(x_d, b: int, h: int, ho0: int, di: int, lt_load: int):
    """Source AP for one di of the unfold load: (dj, c, l) nesting matching
    target partitions (di*3+dj)*8 + c, free dim = padded l' = ho*64+wo."""
    ap = x_d[b, h * 8 : (h + 1) * 8, ho0 + di, 0:3].unsqueeze(-1)
    v = ap.ap
    v[0] = [1, 3]
    v[1] = [H * WIDTH, 8]
    v[2] = [1, lt_load]
    return ap


def build_nc(reps: int = 1, skew: int = 1, warmups: int = 36):
    """Build the per-core program.  reps>1 wraps the body in an on-chip loop
    (used only for device-time measurement); skew is the software-pipeline
    depth between a tile's selection/squares and its main matmuls."""
    f32, f16 = mybir.dt.float32, mybir.dt.float16
    nc = bacc.Bacc("TRN2", target_bir_lowering=False)
    x_d = nc.dram_tensor("x_loc", [B_LOC, C, H, WIDTH], f16, kind="ExternalInput")
    a_d = nc.dram_tensor("aselT", [72, GH], f16, kind="ExternalInput")
    w_d = nc.dram_tensor("w2T", [2 * GH, O], f16, kind="ExternalInput")
    o_d = nc.dram_tensor("out_loc", [B_LOC, O, HO, WO], f32, kind="ExternalOutput")

    with TileContext(nc) as tc:
        with (
            tc.tile_pool(name="const", bufs=1) as cpool,
            tc.tile_pool(name="xin", bufs=2 * B_LOC) as xpool,
            tc.tile_pool(name="gbuf", bufs=3 * (skew + 1) + 3) as gpool,
            tc.tile_pool(name="tmpbuf", bufs=4) as tmppool,
            tc.tile_pool(name="obuf", bufs=6) as opool,
            tc.tile_pool(name="ps_sel", bufs=3, space="PSUM") as pspool,
            tc.tile_pool(name="ps_out", bufs=2, space="PSUM") as popool,
        ):
            LFULL = HO * 64  # 3968 columns of the padded l' = ho*64+wo axis

            # warmup tile first: memset has no DMA dependency, so dummy
            # matmuls can start immediately and hold the HAM clock gate
            # open (2.4 GHz) through the ~9us DMA-engine start-up window
            wt = cpool.tile([GC, 512], f16, tag="warm")
            nc.vector.memset(wt[:], 0.0)

            a_r = cpool.tile([72, GH], f16, tag="a_r")
            nc.sync.dma_start(a_r[:], a_d[:])

            def load_x(x_t, b, h, col0, col1, eng=None):
                """Fill x_t[:, col0:col1] of the unfold view for (b, c-half h)."""
                eng = eng or nc.sync
                for di in range(3):
                    hi = min(col1, H * WIDTH - di * 64 - 2)
                    if hi > col0:
                        ap = _x_window_ap(x_d, b, h, 0, di, hi - col0)
                        ap.offset += col0
                        eng.dma_start(x_t[di * 24 : (di + 1) * 24, col0:hi], ap)
                    if hi < col1:
                        # pad columns feed discarded outputs; fill with
                        # arbitrary valid f32r data to keep reads clean
                        eng.dma_start(
                            x_t[di * 24 : (di + 1) * 24, hi:col1],
                            _x_window_ap(x_d, b, h, 0, 0, col1 - hi),
                        )

            # all unfold loads up front; batch 0 split so tile 0 starts
            # early; the two halves go to different DMA queues (sync /
            # scalar) and batch 1's second half to gpsimd so transfers
            # overlap across engine sets.  All four x tiles are resident
            # (bufs=4), so batch 1 never waits on batch 0's buffers.
            xr_all = []
            for b in range(B_LOC):
                xr_b = []
                for h in range(2):
                    x_t = xpool.tile([72, LFULL], f16, tag=f"x{h}", name=f"x{h}_{b}")
                    xr_b.append(x_t)
                xr_all.append(xr_b)
            load_x(xr_all[0][0], 0, 0, 0, 1024, nc.sync)
            load_x(xr_all[0][1], 0, 1, 0, 1024, nc.scalar)
            load_x(xr_all[0][0], 0, 0, 2496, LFULL, nc.gpsimd)
            load_x(xr_all[0][1], 0, 1, 2496, LFULL, nc.gpsimd)
            load_x(xr_all[0][0], 0, 0, 1024, 2496, nc.scalar)
            load_x(xr_all[0][1], 0, 1, 1024, 2496, nc.sync)
            w_r = cpool.tile([GC, NCHUNK, O], f16, tag="w_r")
            nc.sync.dma_start(w_r[:], w_d[:].rearrange("(k p) o -> p k o", p=GC))
            for b in range(1, B_LOC):
                load_x(xr_all[b][0], b, 0, 0, LFULL, nc.sync)
                load_x(xr_all[b][1], b, 1, 0, LFULL, nc.gpsimd)

            # greedy ACT/DVE load balancing for PSUM-draining elementwise
            # ops (DVE pays double for squares: bounce + SBUF square)
            eng_busy = {"act": 0.0, "dve": 0.0}

            def square_merged(g_t, ps_s, lt):
                gv = g_t[:, :, :lt]
                pv = ps_s[:, :, :lt]
                if eng_busy["act"] + 1.0 <= eng_busy["dve"] + 1.75:
                    nc.scalar.square(gv, pv)
                    eng_busy["act"] += 1.0
                else:
                    # f16 bounce: PSUM f32 read is 1x either way, but the
                    # f16 x f16 multiply runs at DVE 2x rate
                    tmp = tmppool.tile([GC, 2, 512], f16, tag="sq_tmp")
                    tv = tmp[:, :, :lt]
                    nc.vector.tensor_copy(tv, pv)
                    nc.vector.tensor_mul(gv, tv, tv)
                    eng_busy["dve"] += 1.75

            def out_copy(o_view, ps_view):
                if eng_busy["act"] + 0.9 < eng_busy["dve"] + 0.55:
                    nc.scalar.copy(o_view, ps_view)
                    eng_busy["act"] += 0.9
                else:
                    nc.vector.tensor_copy(o_view, ps_view)
                    eng_busy["dve"] += 0.55

            def do_mains(st):
                """Main matmuls + drain for a tile whose squares are issued."""
                b, ho0, nr, g_ts = st
                lt = nr * 64
                ps_o = popool.tile([O, 512], f32, tag="ps_o", name="ps_o")
                for kk in range(NCHUNK):
                    nc.tensor.matmul(
                        ps_o[:, :lt],
                        w_r[:, kk, :],
                        g_ts[kk // 2][:, kk % 2, :lt],
                        start=(kk == 0),
                        stop=(kk == NCHUNK - 1),
                    )
                # compact to [O, nr*62] so the store uses contiguous chunks
                o_t = opool.tile([O, 8 * WO], f32, tag="o", name="o_t")
                ps_view = ps_o[:, :lt].rearrange("o (r w) -> o r w", w=64)
                o_view = o_t[:, : nr * WO].rearrange("o (r w) -> o r w", w=WO)
                out_copy(o_view, ps_view[:, :, :WO])
                nc.gpsimd.dma_start(
                    o_d[b, :, ho0 : ho0 + nr, :],
                    o_t[:, : nr * WO],
                )

            # HAM warmup: keep the PE busy from t=0 through the DMA start-up
            # window so the clock gate is at 8/8 when real matmuls start
            # (dummy MMs on a memset tile; outputs never read)
            def warmup():
                for i in range(warmups):
                    ps_w = popool.tile([O, 512], f32, tag="ps_o", name="warm")
                    nc.tensor.matmul(
                        ps_w[:], wt[:, :GC], wt[:], start=True, stop=True
                    )

            def body(it=None, unroll=1):
                # software-pipeline skew: issue tile t's selections and
                # squares, then tile (t-skew)'s mains — squares get `skew`
                # tiles of slack before the PE needs their output
                pending = []
                for b in range(B_LOC):
                    xr = xr_all[b]
                    for ho0, nr in ROW_TILES:
                        lt = nr * 64
                        c0 = ho0 * 64
                        g_ts = []
                        for kp in range(NCHUNK // 2):
                            # two 120-row chunks share one 2-bank PSUM tile so
                            # one elementwise op drains both
                            ps_s = pspool.tile(
                                [GC, 2, 512], f32, tag="ps_s", name="ps_s"
                            )
                            for half in range(2):
                                kk = kp * 2 + half
                                h, k = divmod(kk, 3)
                                nc.tensor.matmul(
                                    ps_s[:, half, :lt],
                                    a_r[:, k * GC : (k + 1) * GC],
                                    xr[h][:, c0 : c0 + lt],
                                    start=True,
                                    stop=True,
                                )
                            g_t = gpool.tile(
                                [GC, 2, 512], f16, tag="g", name="g_t"
                            )
                            square_merged(g_t, ps_s, lt)
                            g_ts.append(g_t)
                        pending.append((b, ho0, nr, g_ts))
                        if len(pending) > skew:
                            do_mains(pending.pop(0))
                for st in pending:
                    do_mains(st)

            warmup()
            if reps == 1:
                body()
            else:
                hint = (
                    mybir.EngineType.PE,
                    mybir.EngineType.Activation,
                    mybir.EngineType.DVE,
                    mybir.EngineType.SP,
                    mybir.EngineType.Pool,
                )
                with tc.For_i(0, reps, 1, hint_engines=hint) as _it:
                    body()
    nc.compile()
    return nc


@functools.lru_cache(maxsize=1)
def _cached_nc():
    return build_nc()


def _core_inputs(x: np.ndarray, consts, k: int) -> dict:
    AselT, W2T = consts
    x_r = np.asarray(x, dtype=np.float32).astype(np.float16)
    return {
        "x_loc": np.ascontiguousarray(x_r[k * B_LOC : (k + 1) * B_LOC]),
        "aselT": AselT,
        "w2T": W2T,
    }


def kernel(x: np.ndarray, W: np.ndarray, _trace: bool = False):
    x = np.asarray(x, dtype=np.float32)
    W = np.asarray(W, dtype=np.float32)
    consts = _build_consts(W)

    nc = _cached_nc()
    in_maps = [_core_inputs(x, consts, k) for k in range(N_CORES)]
    try:
        r = run_bass_kernel_spmd(
            nc, in_maps, core_ids=list(range(N_CORES)), trace=_trace
        )
    except Exception:
        # transient NRT_EXEC_UNIT_UNRECOVERABLE has been observed once on
        # this fabric; a fresh attempt recovers
        r = run_bass_kernel_spmd(
            nc, in_maps, core_ids=list(range(N_CORES)), trace=_trace
        )
    out = np.concatenate([m["out_loc"] for m in r.results], axis=0)
    if _trace:
        kernel.last_result = r
    return out


if __name__ == "__main__":
    rng = np.random.default_rng(0)
    x = rng.standard_normal((B, C, H, WIDTH), dtype=np.float32)
    W = rng.standard_normal((O, C * 81), dtype=np.float32)
    out = kernel(x, W)
    print("out shape", out.shape, out.dtype)



# revision 25
# speedup vs baseline: 2.0038x; 1.1117x over previous
"""Trainium2 Bass kernel for nn_Fast2Order_DE_Conv.

Math: out[b,o,ho,wo] = sum_{c,i,j} W[o, c*81+i*9+j] * p_i * p_j with
p_i = x[b, c, ho+di, wo+dj] (i = di*3+dj, 3x3 unfold of a 16-channel 64x64
image; output 62x62).

Algorithm: change the quadratic-feature basis from products p_i*p_j to
squares {p_i^2, (p_i+p_j)^2, i<j} (45 per channel, 720 total) and fold the
basis change into W on the host (W2 = W * M^-1).  On-chip, per spatial tile
of 512 locations:

    selection matmul (PE, f16):  s = AselT.T @ x_unfold  [768 padded rows]
    square          (ACT/DVE):   g = s^2, PSUM -> SBUF f16
    main matmul     (PE, f16):   out += W2T.T @ g, accumulated in fp32 PSUM

All matmuls use float16 (e5m10: ~f32r accuracy at half the width, 2-byte
FWL-eligible weight loads, full PE rate).  Inputs are cast to f16 on the
host so DMA loads feed the PE directly.  The 3x3 unfold itself is free: it
is expressed in the DMA access pattern (overlapping windows of the padded
l' = ho*64+wo layout).

Pipelining: tiles are software-pipelined with skew 3 (a tile's selection
matmuls + squares issue three tiles before its main matmuls) so the PE
never waits on the square engines; a burst of warmup matmuls during the
initial DMA window keeps the PE clock gate at full rate.  Per-core device
time ~55 us, ~80% of the PE streaming roofline; the residue is per-matmul
weight-load and dispatch overhead.

Sharding: data-parallel over batch, 2 batches per core on 8 cores; W-side
constants are replicated.  Output gathered by simple concatenation.
"""

import functools

import numpy as np

import concourse.bacc as bacc
import concourse.mybir as mybir
from concourse.tile import TileContext
from concourse.bass_utils import run_bass_kernel_spmd

B, C, H, WIDTH = 16, 16, 64, 64
O = 128
HO = WO = 62
N_CORES = 8
B_LOC = B // N_CORES
PAIRS = [(i, j) for i in range(9) for j in range(i, 9)]  # 45
ROW_TILES = [(0, 8), (8, 8), (16, 8), (24, 8), (32, 8), (40, 8), (48, 8), (56, 6)]
NCHUNK = 6  # g chunks of 128 rows (768 total, 48 zero-padded)
GC = 128
GH = 384  # padded g rows per c-half (360 real + 24 pad)


def _round_f32r(a: np.ndarray) -> np.ndarray:
    """Round fp32 values to the f32r grid (RNE at 12 low mantissa bits)."""
    a = np.ascontiguousarray(a, dtype=np.float32)
    bits = a.view(np.uint32).astype(np.uint64)
    half, mask = np.uint64(0x800), np.uint64(0xFFF)
    lsb = (bits >> np.uint64(12)) & np.uint64(1)
    out = ((bits + half - np.uint64(1) + lsb) & ~mask).astype(np.uint32)
    return out.view(np.float32).reshape(a.shape)


def _build_consts(Wf: np.ndarray):
    """W (128, 1296) -> (AselT [72, 360] f32, W2T [720, 128] f32, f32r grid)."""
    Wt = np.asarray(Wf, dtype=np.float64).reshape(O, C, 9, 9)
    Wsym = Wt + Wt.transpose(0, 1, 3, 2)
    W2 = np.zeros((O, 720))
    for c in range(C):
        for pi, (i, j) in enumerate(PAIRS):
            f = c * 45 + pi
            if i == j:
                W2[:, f] = Wt[:, c, i, i] - 0.5 * (
                    Wsym[:, c, i, :].sum(-1) - 2.0 * Wt[:, c, i, i]
                )
            else:
                W2[:, f] = 0.5 * Wsym[:, c, i, j]
    # x-row layout on chip: row = i*8 + c_local (i = di*3+dj kernel position)
    AselT = np.zeros((72, 384), dtype=np.float32)
    for cl in range(8):
        for pi, (i, j) in enumerate(PAIRS):
            g = cl * 45 + pi
            AselT[i * 8 + cl, g] += 1.0
            if i != j:
                AselT[j * 8 + cl, g] += 1.0
    # pad each c-half's 360 features to 384 (3 chunks of 128) so every
    # selection matmul has exactly 128 stationary columns (enables FWL)
    W2p = np.zeros((O, 768))
    W2p[:, 0:360] = W2[:, 0:360]
    W2p[:, 384:744] = W2[:, 360:720]
    W2T = np.ascontiguousarray(W2p.T).astype(np.float16)  # [768, 128]
    return AselT.astype(np.float16), W2T


def _x_window_ap(x_d, b: int, h: int, ho0: int, di: int, lt_load: int):
    """Source AP for one di of the unfold load: (dj, c, l) nesting matching
    target partitions (di*3+dj)*8 + c, free dim = padded l' = ho*64+wo."""
    ap = x_d[b, h * 8 : (h + 1) * 8, ho0 + di, 0:3].unsqueeze(-1)
    v = ap.ap
    v[0] = [1, 3]
    v[1] = [H * WIDTH, 8]
    v[2] = [1, lt_load]
    return ap


def build_nc(reps: int = 1, skew: int = 1, warmups: int = 36):
    """Build the per-core program.  reps>1 wraps the body in an on-chip loop
    (used only for device-time measurement); skew is the software-pipeline
    depth between a tile's selection/squares and its main matmuls."""
    f32, f16 = mybir.dt.float32, mybir.dt.float16
    nc = bacc.Bacc("TRN2", target_bir_lowering=False)
    x_d = nc.dram_tensor("x_loc", [B_LOC, C, H, WIDTH], f16, kind="ExternalInput")
    a_d = nc.dram_tensor("aselT", [72, GH], f16, kind="ExternalInput")
    w_d = nc.dram_tensor("w2T", [2 * GH, O], f16, kind="ExternalInput")
    o_d = nc.dram_tensor("out_loc", [B_LOC, O, HO, WO], f32, kind="ExternalOutput")

    with TileContext(nc) as tc:
        with (
            tc.tile_pool(name="const", bufs=1) as cpool,
            tc.tile_pool(name="xin", bufs=2 * B_LOC) as xpool,
            tc.tile_pool(name="gbuf", bufs=3 * (skew + 1) + 3) as gpool,
            tc.tile_pool(name="tmpbuf", bufs=4) as tmppool,
            tc.tile_pool(name="obuf", bufs=6) as opool,
            tc.tile_pool(name="ps_sel", bufs=3, space="PSUM") as pspool,
            tc.tile_pool(name="ps_out", bufs=2, space="PSUM") as popool,
        ):
            LFULL = HO * 64  # 3968 columns of the padded l' = ho*64+wo axis

            # warmup tile first: memset has no DMA dependency, so dummy
            # matmuls can start immediately and hold the HAM clock gate
            # open (2.4 GHz) through the ~9us DMA-engine start-up window
            wt = cpool.tile([GC, 512], f16, tag="warm")
            nc.vector.memset(wt[:], 0.0)

            a_r = cpool.tile([72, GH], f16, tag="a_r")
            nc.sync.dma_start(a_r[:], a_d[:])

            def load_x(x_t, b, h, col0, col1, eng=None):
                """Fill x_t[:, col0:col1] of the unfold view for (b, c-half h)."""
                eng = eng or nc.sync
                for di in range(3):
                    hi = min(col1, H * WIDTH - di * 64 - 2)
                    if hi > col0:
                        ap = _x_window_ap(x_d, b, h, 0, di, hi - col0)
                        ap.offset += col0
                        eng.dma_start(x_t[di * 24 : (di + 1) * 24, col0:hi], ap)
                    if hi < col1:
                        # pad columns feed discarded outputs; fill with
                        # arbitrary valid f32r data to keep reads clean
                        eng.dma_start(
                            x_t[di * 24 : (di + 1) * 24, hi:col1],
                            _x_window_ap(x_d, b, h, 0, 0, col1 - hi),
                        )

            # all unfold loads up front; batch 0 split so tile 0 starts
            # early; the two halves go to different DMA queues (sync /
            # scalar) and batch 1's second half to gpsimd so transfers
            # overlap across engine sets.  All four x tiles are resident
            # (bufs=4), so batch 1 never waits on batch 0's buffers.
            xr_all = []
            for b in range(B_LOC):
                xr_b = []
                for h in range(2):
                    x_t = xpool.tile([72, LFULL], f16, tag=f"x{h}", name=f"x{h}_{b}")
                    xr_b.append(x_t)
                xr_all.append(xr_b)
            # all x unfold loads go through gpsimd's software-DGE queue: the
            # hardware-DGE rings (sync/scalar) share only ~3 DMA engines
            # (~67 GB/s total -- exactly the consumption rate), while SWDGE
            # spreads over 16 engines.  sync carries the small constants.
            load_x(xr_all[0][0], 0, 0, 0, 1024, nc.gpsimd)
            load_x(xr_all[0][1], 0, 1, 0, 1024, nc.gpsimd)
            w_r = cpool.tile([GC, NCHUNK, O], f16, tag="w_r")
            nc.sync.dma_start(w_r[:], w_d[:].rearrange("(k p) o -> p k o", p=GC))
            load_x(xr_all[0][0], 0, 0, 1024, LFULL, nc.gpsimd)
            load_x(xr_all[0][1], 0, 1, 1024, LFULL, nc.gpsimd)
            for b in range(1, B_LOC):
                load_x(xr_all[b][0], b, 0, 0, LFULL, nc.gpsimd)
                load_x(xr_all[b][1], b, 1, 0, LFULL, nc.gpsimd)

            # greedy ACT/DVE load balancing for PSUM-draining elementwise
            # ops (DVE pays double for squares: bounce + SBUF square)
            eng_busy = {"act": 0.0, "dve": 0.0}

            def square_merged(g_t, ps_s, lt):
                gv = g_t[:, :, :lt]
                pv = ps_s[:, :, :lt]
                if eng_busy["act"] + 1.0 <= eng_busy["dve"] + 1.75:
                    nc.scalar.square(gv, pv)
                    eng_busy["act"] += 1.0
                else:
                    # f16 bounce: PSUM f32 read is 1x either way, but the
                    # f16 x f16 multiply runs at DVE 2x rate
                    tmp = tmppool.tile([GC, 2, 512], f16, tag="sq_tmp")
                    tv = tmp[:, :, :lt]
                    nc.vector.tensor_copy(tv, pv)
                    nc.vector.tensor_mul(gv, tv, tv)
                    eng_busy["dve"] += 1.75

            def out_copy(o_view, ps_view):
                if eng_busy["act"] + 0.9 < eng_busy["dve"] + 0.55:
                    nc.scalar.copy(o_view, ps_view)
                    eng_busy["act"] += 0.9
                else:
                    nc.vector.tensor_copy(o_view, ps_view)
                    eng_busy["dve"] += 0.55

            def do_mains(st):
                """Main matmuls + drain for a tile whose squares are issued."""
                b, ho0, nr, g_ts = st
                lt = nr * 64
                ps_o = popool.tile([O, 512], f32, tag="ps_o", name="ps_o")
                for kk in range(NCHUNK):
                    nc.tensor.matmul(
                        ps_o[:, :lt],
                        w_r[:, kk, :],
                        g_ts[kk // 2][:, kk % 2, :lt],
                        start=(kk == 0),
                        stop=(kk == NCHUNK - 1),
                    )
                # compact to [O, nr*62] so the store uses contiguous chunks
                o_t = opool.tile([O, 8 * WO], f32, tag="o", name="o_t")
                ps_view = ps_o[:, :lt].rearrange("o (r w) -> o r w", w=64)
                o_view = o_t[:, : nr * WO].rearrange("o (r w) -> o r w", w=WO)
                out_copy(o_view, ps_view[:, :, :WO])
                nc.gpsimd.dma_start(
                    o_d[b, :, ho0 : ho0 + nr, :],
                    o_t[:, : nr * WO],
                )

            # HAM warmup: keep the PE busy from t=0 through the DMA start-up
            # window so the clock gate is at 8/8 when real matmuls start
            # (dummy MMs on a memset tile; outputs never read)
            def warmup():
                for i in range(warmups):
                    ps_w = popool.tile([O, 512], f32, tag="ps_o", name="warm")
                    nc.tensor.matmul(
                        ps_w[:], wt[:, :GC], wt[:], start=True, stop=True
                    )

            def body(it=None, unroll=1):
                # software-pipeline skew: issue tile t's selections and
                # squares, then tile (t-skew)'s mains — squares get `skew`
                # tiles of slack before the PE needs their output
                pending = []
                for b in range(B_LOC):
                    xr = xr_all[b]
                    for ho0, nr in ROW_TILES:
                        lt = nr * 64
                        c0 = ho0 * 64
                        g_ts = []
                        for kp in range(NCHUNK // 2):
                            # two 120-row chunks share one 2-bank PSUM tile so
                            # one elementwise op drains both
                            ps_s = pspool.tile(
                                [GC, 2, 512], f32, tag="ps_s", name="ps_s"
                            )
                            for half in range(2):
                                kk = kp * 2 + half
                                h, k = divmod(kk, 3)
                                nc.tensor.matmul(
                                    ps_s[:, half, :lt],
                                    a_r[:, k * GC : (k + 1) * GC],
                                    xr[h][:, c0 : c0 + lt],
                                    start=True,
                                    stop=True,
                                )
                            g_t = gpool.tile(
                                [GC, 2, 512], f16, tag="g", name="g_t"
                            )
                            square_merged(g_t, ps_s, lt)
                            g_ts.append(g_t)
                        pending.append((b, ho0, nr, g_ts))
                        if len(pending) > skew:
                            do_mains(pending.pop(0))
                for st in pending:
                    do_mains(st)

            warmup()
            if reps == 1:
                body()
            else:
                hint = (
                    mybir.EngineType.PE,
                    mybir.EngineType.Activation,
                    mybir.EngineType.DVE,
                    mybir.EngineType.SP,
                    mybir.EngineType.Pool,
                )
                with tc.For_i(0, reps, 1, hint_engines=hint) as _it:
                    body()
    nc.compile()
    return nc


@functools.lru_cache(maxsize=1)
def _cached_nc():
    return build_nc()


def _core_inputs(x: np.ndarray, consts, k: int) -> dict:
    AselT, W2T = consts
    x_r = np.asarray(x, dtype=np.float32).astype(np.float16)
    return {
        "x_loc": np.ascontiguousarray(x_r[k * B_LOC : (k + 1) * B_LOC]),
        "aselT": AselT,
        "w2T": W2T,
    }


def kernel(x: np.ndarray, W: np.ndarray, _trace: bool = False):
    x = np.asarray(x, dtype=np.float32)
    W = np.asarray(W, dtype=np.float32)
    consts = _build_consts(W)

    nc = _cached_nc()
    in_maps = [_core_inputs(x, consts, k) for k in range(N_CORES)]
    try:
        r = run_bass_kernel_spmd(
            nc, in_maps, core_ids=list(range(N_CORES)), trace=_trace
        )
    except Exception:
        # transient NRT_EXEC_UNIT_UNRECOVERABLE has been observed once on
        # this fabric; a fresh attempt recovers
        r = run_bass_kernel_spmd(
            nc, in_maps, core_ids=list(range(N_CORES)), trace=_trace
        )
    out = np.concatenate([m["out_loc"] for m in r.results], axis=0)
    if _trace:
        kernel.last_result = r
    return out


if __name__ == "__main__":
    rng = np.random.default_rng(0)
    x = rng.standard_normal((B, C, H, WIDTH), dtype=np.float32)
    W = rng.standard_normal((O, C * 81), dtype=np.float32)
    out = kernel(x, W)
    print("out shape", out.shape, out.dtype)

